# revision 1
# baseline (speedup 1.0000x reference)
import sys

sys.path.insert(0, "/opt/trn_rl_repo")
import numpy as np

N1, N2, D = 8192, 8192, 256
NCORES = 8
QPC = N1 // NCORES  # queries per core (1024)
RT = QPC // 128  # row tiles per core (8)
GW = 2048  # group width (4 psum banks)
NEG = -1.0e30


def _build_nc():
    import concourse.bass as bass
    import concourse.tile as tile
    from concourse import mybir

    f32, f32r = mybir.dt.float32, mybir.dt.float32r
    nc = bass.Bass()
    db000d = nc.dram_tensor("db000", [128, 2, GW], f32r, kind="ExternalInput")
    db00bd = nc.dram_tensor("db00b", [128, 2, GW], f32r, kind="ExternalInput")
    db01d = nc.dram_tensor("db01", [128, 2, 2 * GW], f32r, kind="ExternalInput")
    db10d = nc.dram_tensor("db10", [128, 2, 2 * GW], f32r, kind="ExternalInput")
    db11d = nc.dram_tensor("db11", [128, 2, 2 * GW], f32r, kind="ExternalInput")
    bf16 = mybir.dt.bfloat16
    nrmA = nc.dram_tensor("nrmA", [2, N1 + 128], bf16, kind="ExternalInput")
    nrmB = nc.dram_tensor("nrmB", [2, N1], bf16, kind="ExternalInput")
    dmask = nc.dram_tensor("dmask", [128, 4 * 512], f32, kind="ExternalInput")
    o = nc.dram_tensor("o", [128, RT, 8], f32, kind="ExternalOutput")

    with tile.TileContext(nc) as tc:
        with (
            tc.tile_pool(name="sb", bufs=1) as sb,
            tc.tile_pool(name="pp", bufs=1) as pp,
            tc.tile_pool(name="ps", bufs=2, space="PSUM") as ps,
        ):
            CW = 2 * GW  # chunk width (4096 cols)
            # small inputs first: the first norm-MM/mask-TT consumers must not
            # queue behind the 16MB database load on the same DMA engines
            tnrA = sb.tile([2, N1 + 128], bf16, tag="nrA")
            tnrB = sb.tile([2, N1], bf16, tag="nrB")
            tmk = sb.tile([128, 4 * 512], f32, tag="mk")
            nc.sync.dma_start(out=tnrA, in_=nrmA[:])
            nc.sync.dma_start(out=tnrB, in_=nrmB[:])
            nc.scalar.dma_start(out=tmk, in_=dmask[:])
            # first chunk split in half across both HWDGE engines so the
            # opening matmul group's data lands ASAP
            t000 = sb.tile([128, 2, GW], f32r, name="db000", tag="db000")
            nc.sync.dma_start(out=t000, in_=db000d[:])
            t00b = sb.tile([128, 2, GW], f32r, name="db00b", tag="db00b")
            nc.scalar.dma_start(out=t00b, in_=db00bd[:])
            chunks = {(0, 0): (t000, t00b)}
            for (side, h), dsrc, eng in [
                ((0, 1), db01d, nc.sync),
                ((1, 0), db10d, nc.scalar),
                ((1, 1), db11d, nc.sync),
            ]:
                t = sb.tile([128, 2, CW], f32r, name=f"db{side}{h}", tag=f"db{side}{h}")
                eng.dma_start(out=t, in_=dsrc[:])
                chunks[(side, h)] = t
            ones2 = tnrA[0:2, N1 : N1 + 128]
            # wait absorber: DVE observes the dmask DMA once, up front
            dum = sb.tile([128, 1], f32, tag="dum")
            nc.vector.tensor_copy(out=dum, in_=tmk[:, 0:1])
            parts = [pp.tile([128, 8], f32, name=f"part{m}", tag=f"part{m}") for m in range(RT)]
            tq = t000
            for side in range(2):
                for h in range(2):
                    tch = chunks[(side, h)]
                    for m in range(RT):
                        lhs = [tq[:, k, m * 128 : (m + 1) * 128] for k in (0, 1)]
                        part = parts[m]
                        for g2 in range(2):
                            g = h * 2 + g2
                            col = g * GW
                            if isinstance(tch, tuple):
                                tsrc, lcol = tch[g2], 0
                            else:
                                tsrc, lcol = tch, g2 * GW
                            pst = ps.tile([128, GW], f32, tag="pst")
                            for k in (0, 1):
                                for i in range(4):
                                    nc.tensor.matmul(
                                        out=pst[:, i * 512 : (i + 1) * 512],
                                        lhsT=lhs[k],
                                        rhs=tsrc[
                                            :, k, lcol + i * 512 : lcol + (i + 1) * 512
                                        ],
                                        start=(k == 0),
                                        stop=False,
                                    )
                            for i in range(4):
                                nc.tensor.matmul(
                                    out=pst[:, i * 512 : (i + 1) * 512],
                                    lhsT=ones2,
                                    rhs=(tnrA if side == 0 else tnrB)[
                                        0:2, col + i * 512 : col + (i + 1) * 512
                                    ],
                                    start=False,
                                    stop=True,
                                )
                            if side == 0 and g == 0:
                                i0, v = m // 4, m % 4
                                sl = pst[:, i0 * 512 : (i0 + 1) * 512]
                                nc.vector.tensor_add(
                                    out=sl, in0=sl, in1=tmk[:, v * 512 : (v + 1) * 512]
                                )
                            nc.vector.tensor_reduce(
                                out=part[:, side * 4 + g : side * 4 + g + 1],
                                in_=pst,
                                axis=mybir.AxisListType.X,
                                op=mybir.AluOpType.max,
                            )
            for m in range(RT):
                nc.sync.dma_start(out=o[:, m, :], in_=parts[m])

    from concourse.bass import _bass_rust

    _bass_rust.move_matmul_waits_to_ldweights(nc.m)
    _bass_rust.generate_event_semaphores(nc)
    return nc


def _bf16_hilo(x32):
    import ml_dtypes
    hi = x32.astype(ml_dtypes.bfloat16)
    lo = (x32 - hi.astype(np.float32)).astype(ml_dtypes.bfloat16)
    return hi, lo


def _prep_core(s1, s2T, sq2hi, sq2lo, c):
    s1p = np.roll(s1, -c * QPC, axis=0)
    dbx = np.empty((128, 2, 2 * N1), dtype=np.float32)
    s1pT = np.ascontiguousarray(s1p.T)
    for k in (0, 1):
        dbx[:, k, 0:N1] = s1pT[k * 128 : (k + 1) * 128]
        dbx[:, k, N1 : 2 * N1] = s2T[k * 128 : (k + 1) * 128]
    CW = 2 * GW
    dbd = {
        "db000": np.ascontiguousarray(dbx[:, :, 0:GW]),
        "db00b": np.ascontiguousarray(dbx[:, :, GW:CW]),
        "db01": np.ascontiguousarray(dbx[:, :, CW : 2 * CW]),
        "db10": np.ascontiguousarray(dbx[:, :, N1 : N1 + CW]),
        "db11": np.ascontiguousarray(dbx[:, :, N1 + CW : N1 + 2 * CW]),
    }
    dbx = dbd
    import ml_dtypes
    nA = (-0.5 * np.square(s1p.astype(np.float64)).sum(1)).astype(np.float32)
    hiA, loA = _bf16_hilo(nA)
    nrmA = np.ones((2, N1 + 128), dtype=ml_dtypes.bfloat16)
    nrmA[0, :N1], nrmA[1, :N1] = hiA, loA
    return dbx, nrmA


def kernel(s1, s2, k):
    assert int(k) == 1
    from concourse.bass_utils import run_bass_kernel_spmd

    s1 = np.asarray(s1, dtype=np.float32)
    s2 = np.asarray(s2, dtype=np.float32)
    nB = (-0.5 * np.square(s2.astype(np.float64)).sum(1)).astype(np.float32)
    hiB, loB = _bf16_hilo(nB)
    nrmB = np.stack([hiB, loB])
    s2T = np.ascontiguousarray(s2.T)
    dmask = np.zeros((128, 4 * 512), dtype=np.float32)
    for v in range(4):
        for p in range(128):
            dmask[p, v * 512 + v * 128 + p] = NEG

    nc = _build_nc()
    in_maps = []
    for c in range(NCORES):
        dbd, nrmA = _prep_core(s1, s2T, hiB, nB - hiB, c)
        in_maps.append({**dbd, "nrmA": nrmA, "nrmB": nrmB, "dmask": dmask})
    import os
    res = run_bass_kernel_spmd(
        nc, in_maps, core_ids=list(range(NCORES)),
        trace=os.environ.get("KBENCH_TRACE") == "1",
    )
    kernel.last_results = res

    # host epilogue (float64): rho/nu from per-group maxes, then the estimator
    sq1 = np.square(s1.astype(np.float64)).sum(1)
    total = 0.0
    for c in range(NCORES):
        part = res.results[c]["o"].astype(np.float64)  # [128, RT, 8]
        maxA = part[:, :, 0:4].max(axis=2)  # [128, RT]
        maxB = part[:, :, 4:8].max(axis=2)
        idx = np.arange(RT)[None, :] * 128 + np.arange(128)[:, None]
        orig = (c * QPC + idx) % N1
        sqx = sq1[orig]
        rho_sq = sqx - 2.0 * maxA
        nu_sq = sqx - 2.0 * maxB
        rho_sq = np.maximum(rho_sq, 1e-20)
        nu_sq = np.maximum(nu_sq, 1e-20)
        total += 0.5 * (np.log(nu_sq) - np.log(rho_sq)).sum()
    base = np.log(N2 / (N1 - 1))
    return np.float32(base + (D / N1) * total)



# revision 10
# speedup vs baseline: 1.0449x; 1.0449x over previous
import sys

sys.path.insert(0, "/opt/trn_rl_repo")
import os

import numpy as np

N1, N2, D = 8192, 8192, 256
NCORES = 8
QPC = N1 // NCORES  # queries per core (1024)
TPS = N1 // 128  # db tiles per side (64)
NTILES = 2 * TPS  # 128 total db tiles (side A = s1, side B = s2)
NQR = 4  # dma pieces (quarters) per (side, chunk)
QRW = N1 // NQR  # 2048 cols per piece
NEG = -1.0e30

# Per-tile drain/merge roles:
#   F  : DVE fused  acc_V = max(psum + nrm, acc_V)   (scalar_tensor_tensor)
#   AV : Act drain (psum + nrm -> tmp), DVE merges acc_V = max(acc_V, tmp)
# (Pool/GPSIMD can't run TensorTensor — walrus engine check rejects it.)
def _role(t):
    return "F" if t % 4 == 2 else "AV"


def _build_nc():
    import concourse.bass as bass
    import concourse.tile as tile
    from concourse import mybir

    f32, f32r = mybir.dt.float32, mybir.dt.float32r
    add, vmax = mybir.AluOpType.add, mybir.AluOpType.max

    nc = bass.Bass()
    # db pieces: a<ch><qr> side A (s1, rolled per core), b<ch><qr> side B (s2)
    dpieces = {}
    for side in "ab":
        for ch in range(2):
            for qr in range(NQR):
                nm = f"{side}{ch}{qr}"
                dpieces[nm] = nc.dram_tensor(nm, [128, QRW], f32r, kind="ExternalInput")
    nrmd = nc.dram_tensor("nrm", [128, NTILES], f32, kind="ExternalInput")
    negid = nc.dram_tensor("negi", [128, 128], f32r, kind="ExternalInput")
    posid = nc.dram_tensor("posi", [128, 128], f32r, kind="ExternalInput")
    o = nc.dram_tensor("o", [128, 2, QPC], f32, kind="ExternalOutput")

    with tile.TileContext(nc) as tc:
        with (
            tc.tile_pool(name="sb", bufs=1) as sb,
            tc.tile_pool(name="tp", bufs=4) as tp,
            tc.tile_pool(name="ps", bufs=4, space="PSUM") as ps,
        ):
            # tiny tensors first so early consumers don't queue behind the db
            tnrm = sb.tile([128, NTILES], f32, tag="nrm")
            nc.sync.dma_start(out=tnrm, in_=nrmd[:])
            tnegi = sb.tile([128, 128], f32r, tag="negi")
            nc.scalar.dma_start(out=tnegi, in_=negid[:])
            tposi = sb.tile([128, 128], f32r, tag="posi")
            nc.scalar.dma_start(out=tposi, in_=posid[:])
            # db pieces: chunk0 on sync queue, chunk1 on scalar queue, in
            # consumption order (quarter 0 of side A holds the queries)
            tp_db = {}
            for side in "ab":
                for qr in range(NQR):
                    for ch, eng in ((0, nc.sync), (1, nc.scalar)):
                        nm = f"{side}{ch}{qr}"
                        t = sb.tile([128, QRW], f32r, name=nm, tag=nm)
                        eng.dma_start(out=t, in_=dpieces[nm][:])
                        tp_db[nm] = t

            accs = {
                k: sb.tile([128, QPC], f32, name=f"acc{k}", tag=f"acc{k}")
                for k in ("VA", "VB")
            }
            first = {k: True for k in accs}

            for t in range(NTILES):
                side = "a" if t < TPS else "b"
                tl = t if t < TPS else t - TPS
                qr, off = tl // 16, (tl % 16) * 128
                w = [tp_db[f"{side}{ch}{qr}"] for ch in range(2)]
                q = [tp_db[f"a{ch}0"] for ch in range(2)]

                pst = ps.tile([128, QPC], f32, tag="pst")
                for h in range(2):
                    nc.tensor.matmul(
                        out=pst[:, h * 512 : (h + 1) * 512],
                        lhsT=w[0][:, off : off + 128],
                        rhs=q[0][:, h * 512 : (h + 1) * 512],
                        start=True,
                        stop=False,
                    )
                if t < 8:
                    # self-pair mask: -1e30 on the diagonal block
                    nc.tensor.matmul(
                        out=pst[:, t * 128 : (t + 1) * 128],
                        lhsT=tnegi[:, 0:128],
                        rhs=tposi[:, 0:128],
                        start=False,
                        stop=False,
                        skip_group_check=True,
                    )
                for h in range(2):
                    nc.tensor.matmul(
                        out=pst[:, h * 512 : (h + 1) * 512],
                        lhsT=w[1][:, off : off + 128],
                        rhs=q[1][:, h * 512 : (h + 1) * 512],
                        start=False,
                        stop=True,
                    )

                role = _role(t)
                S = "A" if side == "a" else "B"
                tgt = "V"
                acc = accs[tgt + S]
                bias = tnrm[:, t : t + 1]
                if first[tgt + S]:
                    first[tgt + S] = False
                    if role == "F":
                        nc.vector.tensor_scalar_add(out=acc, in0=pst, scalar1=bias)
                    else:
                        nc.scalar.add(out=acc, in_=pst, add=bias)
                elif role == "F":
                    nc.vector.scalar_tensor_tensor(
                        out=acc, in0=pst, scalar=bias, in1=acc, op0=add, op1=vmax
                    )
                else:
                    tmp = tp.tile([128, QPC], f32, tag=f"tmp{tgt}")
                    nc.scalar.add(out=tmp, in_=pst, add=bias)
                    nc.vector.tensor_max(out=acc, in0=acc, in1=tmp)

                if t == TPS - 1:
                    nc.sync.dma_start(out=o[:, 0, :], in_=accs["VA"])
            nc.sync.dma_start(out=o[:, 1, :], in_=accs["VB"])

    from concourse.bass import _bass_rust

    _bass_rust.move_matmul_waits_to_ldweights(nc.m)
    _bass_rust.generate_event_semaphores(nc)
    return nc


def kernel(s1, s2, k):
    assert int(k) == 1
    from concourse.bass_utils import run_bass_kernel_spmd

    s1 = np.asarray(s1, dtype=np.float32)
    s2 = np.asarray(s2, dtype=np.float32)

    # [128, 2, N] layouts: db[k, ch, j] = x[j, 128*ch + k]
    s1T = np.ascontiguousarray(s1.T.reshape(2, 128, N1).transpose(1, 0, 2))
    s1T2 = np.concatenate([s1T, s1T], axis=2)  # doubled for per-core roll
    s2T = np.ascontiguousarray(s2.T.reshape(2, 128, N2).transpose(1, 0, 2))

    sq1 = np.square(s1.astype(np.float64)).sum(1)
    sq2 = np.square(s2.astype(np.float64)).sum(1)
    n1h = (-0.5 * sq1).astype(np.float32)
    n1h2 = np.concatenate([n1h, n1h])
    n2h = (-0.5 * sq2).astype(np.float32)

    negi = np.zeros((128, 128), dtype=np.float32)
    np.fill_diagonal(negi, NEG)
    posi = np.eye(128, dtype=np.float32)

    bp = {
        f"b{ch}{qr}": np.ascontiguousarray(s2T[:, ch, qr * QRW : (qr + 1) * QRW])
        for ch in range(2)
        for qr in range(NQR)
    }
    nrmB = n2h.reshape(TPS, 128).T  # [128, 64]

    nc = _build_nc()
    in_maps = []
    for c in range(NCORES):
        r0 = c * QPC
        im = dict(bp)
        for ch in range(2):
            for qr in range(NQR):
                im[f"a{ch}{qr}"] = np.ascontiguousarray(
                    s1T2[:, ch, r0 + qr * QRW : r0 + (qr + 1) * QRW]
                )
        nrmA = np.ascontiguousarray(n1h2[r0 : r0 + N1]).reshape(TPS, 128).T
        im["nrm"] = np.ascontiguousarray(
            np.concatenate([nrmA, nrmB], axis=1)
        )  # [128, 128]
        im["negi"] = negi
        im["posi"] = posi
        in_maps.append(im)

    res = run_bass_kernel_spmd(
        nc,
        in_maps,
        core_ids=list(range(NCORES)),
        trace=os.environ.get("KBENCH_TRACE") == "1",
    )
    kernel.last_results = res

    # host epilogue (float64)
    total = 0.0
    for c in range(NCORES):
        o = res.results[c]["o"].astype(np.float64)  # [128, 2, QPC]
        maxA = o[:, 0, :].max(axis=0)  # [QPC]
        maxB = o[:, 1, :].max(axis=0)
        sqx = sq1[c * QPC : (c + 1) * QPC]
        rho_sq = np.maximum(sqx - 2.0 * maxA, 1e-20)
        nu_sq = np.maximum(sqx - 2.0 * maxB, 1e-20)
        total += 0.5 * (np.log(nu_sq) - np.log(rho_sq)).sum()
    base = np.log(N2 / (N1 - 1))
    return np.float32(base + (D / N1) * total)


# revision 16
# speedup vs baseline: 1.4962x; 1.4318x over previous
import sys

sys.path.insert(0, "/opt/trn_rl_repo")
import os

import numpy as np

N1, N2, D = 8192, 8192, 256
NCORES = 8
QPC = N1 // NCORES  # queries per core (1024)
TPS = N1 // 128  # db tiles per side (64)
NTILES = 2 * TPS  # 128 total db tiles (side A = s1, side B = s2)
NQR = 4  # dma pieces (quarters) per (side, chunk)
QRW = N1 // NQR  # 2048 cols per piece
NEG = -60000.0  # below any real value, finite in fp16

# Per-tile drain/merge roles:
#   F  : DVE fused  acc_V = max(psum + nrm, acc_V)   (scalar_tensor_tensor)
#   AV : Act drain (psum + nrm -> tmp), DVE merges acc_V = max(acc_V, tmp)
# (Pool/GPSIMD can't run TensorTensor — walrus engine check rejects it.)
def _role(t):
    return "F" if t % 4 == 2 else "AV"


def _build_nc():
    import concourse.bass as bass
    import concourse.tile as tile
    from concourse import mybir

    f32, f32r = mybir.dt.float32, mybir.dt.float32r
    f16 = mybir.dt.float16
    add, vmax = mybir.AluOpType.add, mybir.AluOpType.max

    nc = bass.Bass()
    # db pieces: a<ch><qr> side A (s1, rolled per core), b<ch><qr> side B (s2)
    # quarter 0 of side A is split in two (q/r) so the queries + first tiles
    # land fast and the PE can start ~2us in
    dpieces = {}
    for side in "ab":
        for ch in range(2):
            for qr in range(NQR):
                nm = f"{side}{ch}{qr}"
                if side == "a" and qr == 0:
                    for half in "qr":
                        dpieces[nm + half] = nc.dram_tensor(
                            nm + half, [128, QRW // 2], f32r, kind="ExternalInput"
                        )
                else:
                    dpieces[nm] = nc.dram_tensor(
                        nm, [128, QRW], f32r, kind="ExternalInput"
                    )
    nrmd = nc.dram_tensor("nrm", [128, NTILES], f32, kind="ExternalInput")
    negid = nc.dram_tensor("negi", [128, 128], f32r, kind="ExternalInput")
    posid = nc.dram_tensor("posi", [128, 128], f32r, kind="ExternalInput")
    o = nc.dram_tensor("o", [128, 2, QPC], f16, kind="ExternalOutput")

    with tile.TileContext(nc) as tc:
        with (
            tc.tile_pool(name="sb", bufs=1) as sb,
            tc.tile_pool(name="tp", bufs=4) as tp,
            tc.tile_pool(name="ps", bufs=4, space="PSUM") as ps,
        ):
            # All DMA issues go on the sync + gpsimd sequencers: issuing them
            # from scalar/vector stalls those engines' compute behind DGE
            # ring waits. Tiny tensors first.
            tnrm = sb.tile([128, NTILES], f32, tag="nrm")
            nc.sync.dma_start(out=tnrm, in_=nrmd[:])
            tnegi = sb.tile([128, 128], f32r, tag="negi")
            nc.gpsimd.dma_start(out=tnegi, in_=negid[:])
            tposi = sb.tile([128, 128], f32r, tag="posi")
            nc.gpsimd.dma_start(out=tposi, in_=posid[:])
            # db pieces: chunk0 on sync, chunk1 on gpsimd, in consumption
            # order (side A quarter 0 holds the queries and tiles 0-15)
            tp_db = {}
            for side in "ab":
                for qr in range(NQR):
                    for ch, eng in ((0, nc.sync), (1, nc.gpsimd)):
                        nm = f"{side}{ch}{qr}"
                        if side == "a" and qr == 0:
                            for half in "qr":
                                t = sb.tile(
                                    [128, QRW // 2], f32r, name=nm + half, tag=nm + half
                                )
                                eng.dma_start(out=t, in_=dpieces[nm + half][:])
                                tp_db[nm + half] = t
                        else:
                            t = sb.tile([128, QRW], f32r, name=nm, tag=nm)
                            eng.dma_start(out=t, in_=dpieces[nm][:])
                            tp_db[nm] = t

            accs = {
                k: sb.tile([128, QPC], f16, name=f"acc{k}", tag=f"acc{k}")
                for k in ("VA", "VB")
            }
            first = {k: True for k in accs}

            for t in range(NTILES):
                side = "a" if t < TPS else "b"
                tl = t if t < TPS else t - TPS
                qr, off = tl // 16, (tl % 16) * 128
                if side == "a" and qr == 0:
                    half = "q" if off < QRW // 2 else "r"
                    w = [tp_db[f"a{ch}0{half}"] for ch in range(2)]
                    if half == "r":
                        off -= QRW // 2
                else:
                    w = [tp_db[f"{side}{ch}{qr}"] for ch in range(2)]
                q = [tp_db[f"a{ch}0q"] for ch in range(2)]

                pst = ps.tile([128, QPC], f32, tag="pst")
                for h in range(2):
                    nc.tensor.matmul(
                        out=pst[:, h * 512 : (h + 1) * 512],
                        lhsT=w[0][:, off : off + 128],
                        rhs=q[0][:, h * 512 : (h + 1) * 512],
                        start=True,
                        stop=False,
                    )
                if t < 8:
                    # self-pair mask: -1e30 on the diagonal block
                    nc.tensor.matmul(
                        out=pst[:, t * 128 : (t + 1) * 128],
                        lhsT=tnegi[:, 0:128],
                        rhs=tposi[:, 0:128],
                        start=False,
                        stop=False,
                        skip_group_check=True,
                    )
                for h in range(2):
                    nc.tensor.matmul(
                        out=pst[:, h * 512 : (h + 1) * 512],
                        lhsT=w[1][:, off : off + 128],
                        rhs=q[1][:, h * 512 : (h + 1) * 512],
                        start=False,
                        stop=True,
                    )

                role = _role(t)
                S = "A" if side == "a" else "B"
                tgt = "V"
                acc = accs[tgt + S]
                bias = tnrm[:, t : t + 1]
                if first[tgt + S]:
                    first[tgt + S] = False
                    if role == "F":
                        nc.vector.tensor_scalar_add(out=acc, in0=pst, scalar1=bias)
                    else:
                        nc.scalar.add(out=acc, in_=pst, add=bias)
                elif role == "F":
                    nc.vector.scalar_tensor_tensor(
                        out=acc, in0=pst, scalar=bias, in1=acc, op0=add, op1=vmax
                    )
                else:
                    tmp = tp.tile([128, QPC], f16, tag=f"tmp{tgt}")
                    nc.scalar.add(out=tmp, in_=pst, add=bias)
                    nc.vector.tensor_max(out=acc, in0=acc, in1=tmp)

                if t == TPS - 1:
                    nc.sync.dma_start(out=o[:, 0, :], in_=accs["VA"])
            nc.sync.dma_start(out=o[:, 1, :], in_=accs["VB"])

    from concourse.bass import _bass_rust

    _bass_rust.move_matmul_waits_to_ldweights(nc.m)
    _bass_rust.generate_event_semaphores(nc)
    return nc


def kernel(s1, s2, k):
    assert int(k) == 1
    from concourse.bass_utils import run_bass_kernel_spmd

    s1 = np.asarray(s1, dtype=np.float32)
    s2 = np.asarray(s2, dtype=np.float32)

    # [128, 2, N] layouts: db[k, ch, j] = x[j, 128*ch + k]
    s1T = np.ascontiguousarray(s1.T.reshape(2, 128, N1).transpose(1, 0, 2))
    s1T2 = np.concatenate([s1T, s1T], axis=2)  # doubled for per-core roll
    s2T = np.ascontiguousarray(s2.T.reshape(2, 128, N2).transpose(1, 0, 2))

    sq1 = np.square(s1.astype(np.float64)).sum(1)
    sq2 = np.square(s2.astype(np.float64)).sum(1)
    n1h = (-0.5 * sq1).astype(np.float32)
    n1h2 = np.concatenate([n1h, n1h])
    n2h = (-0.5 * sq2).astype(np.float32)

    negi = np.zeros((128, 128), dtype=np.float32)
    np.fill_diagonal(negi, NEG)
    posi = np.eye(128, dtype=np.float32)

    bp = {
        f"b{ch}{qr}": np.ascontiguousarray(s2T[:, ch, qr * QRW : (qr + 1) * QRW])
        for ch in range(2)
        for qr in range(NQR)
    }
    nrmB = n2h.reshape(TPS, 128).T  # [128, 64]

    nc = _build_nc()
    in_maps = []
    for c in range(NCORES):
        r0 = c * QPC
        im = dict(bp)
        for ch in range(2):
            for qr in range(NQR):
                piece = np.ascontiguousarray(
                    s1T2[:, ch, r0 + qr * QRW : r0 + (qr + 1) * QRW]
                )
                if qr == 0:
                    im[f"a{ch}0q"] = np.ascontiguousarray(piece[:, : QRW // 2])
                    im[f"a{ch}0r"] = np.ascontiguousarray(piece[:, QRW // 2 :])
                else:
                    im[f"a{ch}{qr}"] = piece
        nrmA = np.ascontiguousarray(n1h2[r0 : r0 + N1]).reshape(TPS, 128).T
        im["nrm"] = np.ascontiguousarray(
            np.concatenate([nrmA, nrmB], axis=1)
        )  # [128, 128]
        im["negi"] = negi
        im["posi"] = posi
        in_maps.append(im)

    res = run_bass_kernel_spmd(
        nc,
        in_maps,
        core_ids=list(range(NCORES)),
        trace=os.environ.get("KBENCH_TRACE") == "1",
    )
    kernel.last_results = res

    # host epilogue (float64)
    total = 0.0
    for c in range(NCORES):
        o = res.results[c]["o"].astype(np.float64)  # [128, 2, QPC]
        maxA = o[:, 0, :].max(axis=0)  # [QPC]
        maxB = o[:, 1, :].max(axis=0)
        sqx = sq1[c * QPC : (c + 1) * QPC]
        rho_sq = np.maximum(sqx - 2.0 * maxA, 1e-20)
        nu_sq = np.maximum(sqx - 2.0 * maxB, 1e-20)
        total += 0.5 * (np.log(nu_sq) - np.log(rho_sq)).sum()
    base = np.log(N2 / (N1 - 1))
    return np.float32(base + (D / N1) * total)
